# revision 11
# baseline (speedup 1.0000x reference)
"""Trainium2 Bass kernel for nn_ExpandedSchedule (ODE schedule solver).

Coarse-chain algorithm:
- The 6x6 per-step transform block-decomposes into a 2x2 block (alpha,lam)
  and a 3x3 block (beta,kappa,nu) that is exactly the symmetric square of
  the 2x2 (kappa = 2 * offdiag, C0_eff uses kappa0/2); component 5 and the
  whole g-MLP never reach the output and are dropped.
- f, r are smooth scalar functions of t and dt=5e-6, so one 2x2 transform
  per 196-step chain, T_c = I + (t_end - t_start) * M(t_mid), approximates
  the per-step Euler product to ~1e-9; the 7 outputs are computed at chain
  boundaries and linearly interpolated to the fine grid (curvature error
  ~1e-7).
- Every core evaluates the MLP at all 8*128 chain midpoints (1024 points)
  and computes every core's chain-product total locally: no collective.

Layout: chains are REVERSED onto partitions (partition p = chain 127-p,
shift matrices eye(k=-d)) so the core-total lands on partition 0, where
the whole 8-core carry (free-dim mini-scan over core totals, carry
quadratic forms, mask-select) runs as tiny single-partition DVE ops; one
PE matmul against an all-ones-row-0 matrix broadcasts the carry row to
all 128 partitions. No DRAM bounces for the carry. Small constants are
host-packed into one [128, NMEGA] parameter; tmids/dtsum are chain-major
so the transform-entry readback is strided-contiguous; activation tables
(Gelu, Ln) are prefetched so their loads overlap DMA / MLP phases.
"""

import sys
for _p in ("/opt/trn_rl_repo", "/root/.axon_site/_ro/trn_rl_repo"):
    if _p not in sys.path:
        sys.path.insert(0, _p)

import numpy as np

import concourse.bass as bass
import concourse.mybir as mybir
import concourse.tile as tile
from concourse.bass_utils import run_bass_kernel_spmd

F32 = mybir.dt.float32
F32R = mybir.dt.float32r
AF = mybir.ActivationFunctionType
ALU = mybir.AluOpType

T = 200001
N = T - 1
NCORES = 8
PER = N // NCORES            # 25000
CH = 128                     # chains per core (one per partition)
L = 196                      # fine steps per chain
NPTS = NCORES * CH           # 1024 MLP points (all cores' midpoints)

# wsml (early small weights param) column offsets
W_W3 = 0          # [128, 4]
W_W1 = 4          # [128, 2]
W_B1 = 6          # [128, 2]
W_B2 = 8          # [128, 2]
W_B3 = 10         # rows 0-1
W_AD = 11         # rows 0-1: (0, 1) adder for entry build
NWSML = 12

# mega-const column offsets
O_C0 = 0          # row 0: (beta0, kappa0/2, nu0)
O_MSK = 3         # [128, 8] one-hot my core (all rows)
O_WP = 11         # [128, 196] lerp weights (row p = chain 127-p)
O_E0 = 207        # [128, 128] all-ones row 0 (partition-0 broadcast)
O_SH = 335        # [128, 7*128] shift matrices eye(k=-2^di)
O_ID = 1231       # [128, 7*32] identity pads (rows >= 128-d)
NMEGA = 1455


def _combine22(nc, pool, A, B, out, eng0=None, eng1=None):
    """out = A @ B on flattened 2x2 entry views [P, nb, 4] (row-major)."""
    P, nb = A.shape[0], A.shape[1]
    eng0 = eng0 or nc.vector
    eng1 = eng1 or eng0
    A4 = A.rearrange("p b (i k) -> p b i k", i=2)
    B4 = B.rearrange("p b (k j) -> p b k j", k=2)
    O4 = out.rearrange("p b (i j) -> p b i j", i=2)
    ts = [pool.tile([128, nb, 2, 2], F32, tag=f"c22_{i}", name=f"c22_{i}")
          for i in range(2)]
    for k in range(2):
        ak = A4[:, :, :, k].unsqueeze(3).broadcast_to([P, nb, 2, 2])
        bk = B4[:, :, k, :].unsqueeze(2).broadcast_to([P, nb, 2, 2])
        (eng0 if k == 0 else eng1).tensor_mul(out=ts[k][:P, :, :, :],
                                              in0=ak, in1=bk)
    eng0.tensor_add(out=O4, in0=ts[0][:P, :, :, :], in1=ts[1][:P, :, :, :])


def _hoist_matmul_waits(nc):
    """This walrus codegen allows only one sync wait per engine instruction;
    move extra waits onto inserted same-engine NoOps just before it."""
    for fn in nc.m.functions:
        for bb in fn.blocks:
            new = []
            for ins in bb.instructions:
                si = getattr(ins, "sync_info", None)
                if (si is not None and si.on_wait and len(si.on_wait) > 1
                        and getattr(ins, "engine", None) is not None):
                    waits = list(si.on_wait)
                    si.on_wait = [waits.pop()]
                    for wi, w in enumerate(waits):
                        new.append(mybir.InstNoOp(
                            name=f"{ins.name}-wgate{wi}", engine=ins.engine,
                            ins=[], outs=[],
                            sync_info=mybir.SyncInfo(on_wait=[w],
                                                     on_update=[])))
                new.append(ins)
            bb.instructions = new


def build_program():
    nc = bass.Bass()

    tmid_d = nc.declare_dram_parameter("tmids", [NPTS], F32, isOutput=False)
    dts_d = nc.declare_dram_parameter("dtsum", [NPTS], F32, isOutput=False)
    w2_d = nc.declare_dram_parameter("w2p", [128, 512], F32, isOutput=False)
    w3_d = nc.declare_dram_parameter("w3p", [128, 4], F32, isOutput=False)
    ws_d = nc.declare_dram_parameter("wsml", [128, NWSML], F32,
                                     isOutput=False)
    mg_d = nc.declare_dram_parameter("mega", [128, NMEGA], F32,
                                     isOutput=False)
    out_d = nc.declare_dram_parameter("out", [CH, L * 7], F32, isOutput=True)

    with tile.TileContext(nc) as tc:
        with (
            tc.tile_pool(name="const", bufs=1) as cp,
            tc.tile_pool(name="dram", bufs=1, space="DRAM") as dp,
            tc.tile_pool(name="work", bufs=1) as wk,
            tc.tile_pool(name="sc2", bufs=1) as sc2,
            tc.tile_pool(name="lvl", bufs=2) as lvl,
            tc.tile_pool(name="ps", bufs=1, space="PSUM") as psp,
            tc.tile_pool(name="ps2", bufs=2, space="PSUM") as ps2,
            tc.tile_pool(name="ps4", bufs=2, space="PSUM") as ps4,
            tc.tile_pool(name="ps1", bufs=1, space="PSUM") as ps1,
            tc.tile_pool(name="sm", bufs=2) as sm,
        ):
            V = nc.vector
            G = nc.gpsimd

            # gelu-table prefetch before any DMA lands
            tiny = cp.tile([1, 1], F32)
            V.memset(tiny[:, :], 0.5)
            tinyo = cp.tile([1, 1], F32)
            nc.scalar.activation(out=tinyo[:, :], in_=tiny[:, :],
                                 func=AF.Gelu, bias=0.0, scale=1.0)

            # ---- input DMAs (MLP-critical first) ----
            tmb = cp.tile([128, NPTS], F32)
            nc.sync.dma_start(
                out=tmb[:, :],
                in_=tmid_d[:].unsqueeze(0).broadcast_to([128, NPTS]))
            wssb = cp.tile([128, NWSML], F32)
            nc.sync.dma_start(out=wssb[:, :], in_=ws_d[:, :])
            w2sb = cp.tile([128, 512], F32R)
            nc.sync.dma_start(out=w2sb[:, :],
                              in_=w2_d[:, :].bitcast(F32R))
            w3sb = cp.tile([128, 4], F32R)
            nc.sync.dma_start(out=w3sb[:, :],
                              in_=w3_d[:, :].bitcast(F32R))
            dt2 = cp.tile([2, NPTS], F32)
            nc.sync.dma_start(
                out=dt2[:, :],
                in_=dts_d[:].unsqueeze(0).broadcast_to([2, NPTS]))
            mg = cp.tile([128, NMEGA], F32)
            nc.sync.dma_start(out=mg[:, :], in_=mg_d[:, :])

            w1col = wssb[:, W_W1:W_W1 + 2]
            b1sb = wssb[:, W_B1:W_B1 + 2]
            b2sb = wssb[:, W_B2:W_B2 + 2]
            b3col = wssb[0:2, W_B3:W_B3 + 1]
            adcol = wssb[0:2, W_AD:W_AD + 1]
            cb0 = mg[0:1, O_C0 + 0:O_C0 + 1]
            ck0 = mg[0:1, O_C0 + 1:O_C0 + 2]
            cn0 = mg[0:1, O_C0 + 2:O_C0 + 3]
            msk = mg[:, O_MSK:O_MSK + 8]
            wp = mg[:, O_WP:O_WP + L]
            e0bc = mg[:, O_E0:O_E0 + 128]
            shsb = mg[:, O_SH:O_SH + 7 * 128]
            idsb = mg[:, O_ID:O_ID + 7 * 32]

            # ---- phase 1: fr-MLP at all 1024 chain midpoints ----
            # (w3 rows swapped host-side: fr2 row0 = r, row1 = f)
            h1 = [wk.tile([128, NPTS], F32R, tag=f"h1_{mi}",
                          name=f"h1_{mi}") for mi in range(2)]
            h2 = [wk.tile([128, NPTS], F32R, tag=f"h2_{mi}",
                          name=f"h2_{mi}") for mi in range(2)]
            fr2 = wk.tile([2, NPTS], F32, tag="fr2")
            HP = NPTS // 2
            for ti in range(2):
                sl = slice(ti * HP, (ti + 1) * HP)
                for mi in range(2):
                    nc.scalar.activation(out=h1[mi][:, sl], in_=tmb[:, sl],
                                         func=AF.Gelu,
                                         bias=b1sb[:, mi:mi + 1],
                                         scale=w1col[:, mi:mi + 1])
                for mi in range(2):
                    p2 = ps2.tile([128, HP], F32, tag=f"p2_{mi}",
                                  name=f"p2_{mi}_{ti}")
                    for kt in range(2):
                        lhs = w2sb[:, kt * 256 + mi * 128:
                                   kt * 256 + (mi + 1) * 128]
                        nc.tensor.matmul(out=p2[:, :], lhsT=lhs,
                                         rhs=h1[kt][:, sl],
                                         start=(kt == 0), stop=(kt == 1))
                    nc.scalar.activation(out=h2[mi][:, sl], in_=p2[:, :],
                                         func=AF.Gelu,
                                         bias=b2sb[:, mi:mi + 1], scale=1.0)
                p3 = ps2.tile([2, HP], F32, tag="p2_0", name=f"p3_{ti}")
                for kt in range(2):
                    nc.tensor.matmul(out=p3[:, :],
                                     lhsT=w3sb[:, 2 * kt:2 * kt + 2],
                                     rhs=h2[kt][:, sl],
                                     start=(kt == 0), stop=(kt == 1))
                nc.vector.tensor_scalar_add(out=fr2[:, sl], in0=p3[:, :],
                                            scalar1=b3col[:, :])

            # Ln-table prefetch; input h2[1] forces it after the last gelu
            nc.scalar.activation(out=tinyo[:, :], in_=h2[1][0:1, 0:1],
                                 func=AF.Ln, bias=tiny[:, :], scale=0.0)

            # ---- phase 2: entry rows: row0 = -dts*r = T01,
            #      row1 = 1 - dts*f = T11 ----
            dtfr = wk.tile([2, NPTS], F32, tag="dtfr")
            V.tensor_mul(out=dtfr[:, :], in0=dt2[:, :], in1=fr2[:, :])
            ent = wk.tile([2, NPTS], F32, tag="ent")
            V.tensor_scalar(out=ent[:, :], in0=dtfr[:, :],
                            scalar1=-1.0, scalar2=adcol[:, :],
                            op0=ALU.mult, op1=ALU.add)
            entd = dp.tile([2, NPTS], F32)
            nc.sync.dma_start(out=entd[:, :], in_=ent[:, :])

            # ---- phase 3: Tc [128, 8 cores, 4]; partition p = chain 127-p
            #      (host already stores tmids/dtsum with reversed chains) ----
            Tc = wk.tile([128, 8, 4], F32, tag="Tc")
            Tcv = Tc[:, :, :]
            Tc4 = Tc.rearrange("p k (a b) -> p k a b", a=2)
            V.memset(Tcv[:, :, 0], 1.0)
            nc.sync.dma_start(
                out=Tc4[:, :, :, 1],
                in_=entd[:, :].rearrange("e (c k) -> c k e", k=8))
            nc.sync.dma_start(
                out=Tcv[:, :, 2],
                in_=dts_d[:].rearrange("(c k) -> c k", k=8))

            # ---- phase 4: joint 7-level Hillis-Steele over partitions ----
            # R_p <- R_p @ R_{p+d} (shift matrices eye(k=-d))
            Rcur = Tc
            for di in range(7):
                pr = psp.tile([128, 32], F32, tag="spr")
                nc.tensor.matmul(out=pr[:, :],
                                 lhsT=shsb[:, di * 128:(di + 1) * 128],
                                 rhs=Rcur[:, :, :].rearrange(
                                     "p k e -> p (k e)"),
                                 start=True, stop=True)
                Bv = lvl.tile([128, 8, 4], F32, tag="Bv", name=f"Bv{di}")
                V.tensor_add(
                    out=Bv[:, :, :],
                    in0=pr[:, :].rearrange("p (k e) -> p k e", k=8),
                    in1=idsb[:, di * 32:(di + 1) * 32].rearrange(
                        "p (k e) -> p k e", k=8))
                Rn = lvl.tile([128, 8, 4], F32, tag="R", name=f"R{di}")
                _combine22(nc, sc2, Rcur[:, :, :], Bv[:, :, :], Rn[:, :, :],
                           eng0=V, eng1=V)
                Rcur = Rn

            # ---- phase 5: carry, entirely on partition 0 ----
            # core totals: Rcur[0, k, :] = full product of core k's chains
            Z = Rcur[0:1, :, :]                     # [1, 8, 4]
            for di in range(3):
                d = 1 << di
                Zn = sm.tile([1, 8, 4], F32, tag="Zn", name=f"Zn{di}")
                G.tensor_copy(out=Zn[:, 0:d, :], in_=Z[:, 0:d, :])
                _combine22(nc, sc2, Z[:, d:8, :], Z[:, 0:8 - d, :],
                           Zn[:, d:8, :], eng0=V)
                Z = Zn[:, :, :]
            # exclusive over cores: Kx[c] = product of cores < c; Kx[0] = I
            Kx = sm.tile([1, 8, 4], F32, tag="Kx")
            V.tensor_copy(out=Kx[:, 1:8, :], in_=Z[:, 0:7, :])
            V.memset(Kx[:, 0, :], 0.0)
            V.memset(Kx[:, 0, 0::3], 1.0)

            # carry quantities Q8 [1, 8 cores, 8]:
            # 0=m0 1=m1 2=cb 3=ck2 4=cn 5=cb2 6=ck2 7=cn2
            kxp = Kx[:, :, 0:1]
            KP = sm.tile([1, 8, 4], F32, tag="KP")   # (pp, pq, pu, pv)
            KQ = sm.tile([1, 8, 4], F32, tag="KQ")   # (qp, qq, qu, qv)
            KU = sm.tile([1, 8, 4], F32, tag="KU")   # (up, uq, uu, uv)
            KV = sm.tile([1, 8, 4], F32, tag="KV")   # (vp, vq, vu, vv)
            V.tensor_mul(out=KP[:, :, :], in0=kxp.broadcast_to([1, 8, 4]),
                         in1=Kx[:, :, :])
            G.tensor_mul(out=KQ[:, :, :],
                         in0=Kx[:, :, 1:2].broadcast_to([1, 8, 4]),
                         in1=Kx[:, :, :])
            V.tensor_mul(out=KU[:, :, :],
                         in0=Kx[:, :, 2:3].broadcast_to([1, 8, 4]),
                         in1=Kx[:, :, :])
            G.tensor_mul(out=KV[:, :, :],
                         in0=Kx[:, :, 3:4].broadcast_to([1, 8, 4]),
                         in1=Kx[:, :, :])
            Q8 = sm.tile([1, 8, 8], F32, tag="Q8")
            V.tensor_copy(out=Q8[:, :, 0], in_=Kx[:, :, 0])
            V.tensor_copy(out=Q8[:, :, 1], in_=Kx[:, :, 2])
            tq = sm.tile([1, 8, 4], F32, tag="tq")
            # cb = pp*cb0 + 2*pq*ck0 + qq*cn0
            V.tensor_scalar_mul(out=tq[:, :, 0], in0=KP[:, :, 0],
                                scalar1=cb0)
            V.tensor_scalar(out=tq[:, :, 1], in0=KP[:, :, 1], scalar1=ck0,
                            scalar2=2.0, op0=ALU.mult, op1=ALU.mult)
            V.tensor_add(out=tq[:, :, 0], in0=tq[:, :, 0], in1=tq[:, :, 1])
            V.tensor_scalar_mul(out=tq[:, :, 1], in0=KQ[:, :, 1],
                                scalar1=cn0)
            V.tensor_add(out=Q8[:, :, 2], in0=tq[:, :, 0], in1=tq[:, :, 1])
            # cn = uu*cb0 + 2*uv*ck0 + vv*cn0
            G.tensor_scalar_mul(out=tq[:, :, 2], in0=KU[:, :, 2],
                                scalar1=cb0)
            G.tensor_scalar(out=tq[:, :, 3], in0=KU[:, :, 3], scalar1=ck0,
                            scalar2=2.0, op0=ALU.mult, op1=ALU.mult)
            G.tensor_add(out=tq[:, :, 2], in0=tq[:, :, 2], in1=tq[:, :, 3])
            G.tensor_scalar_mul(out=tq[:, :, 3], in0=KV[:, :, 3],
                                scalar1=cn0)
            G.tensor_add(out=Q8[:, :, 4], in0=tq[:, :, 2], in1=tq[:, :, 3])
            # ck2 = 2*(pu*cb0 + (pv+qu)*ck0 + qv*cn0)
            tk8 = sm.tile([1, 8, 2], F32, tag="tk8")
            V.tensor_scalar(out=tk8[:, :, 0], in0=KP[:, :, 2], scalar1=cb0,
                            scalar2=2.0, op0=ALU.mult, op1=ALU.mult)
            V.tensor_add(out=tk8[:, :, 1], in0=KP[:, :, 3], in1=KQ[:, :, 2])
            V.tensor_scalar(out=tk8[:, :, 1], in0=tk8[:, :, 1], scalar1=ck0,
                            scalar2=2.0, op0=ALU.mult, op1=ALU.mult)
            V.tensor_add(out=tk8[:, :, 0], in0=tk8[:, :, 0],
                         in1=tk8[:, :, 1])
            V.tensor_scalar(out=tk8[:, :, 1], in0=KQ[:, :, 3], scalar1=cn0,
                            scalar2=2.0, op0=ALU.mult, op1=ALU.mult)
            V.tensor_add(out=Q8[:, :, 3], in0=tk8[:, :, 0],
                         in1=tk8[:, :, 1])
            V.tensor_copy(out=Q8[:, :, 6], in_=Q8[:, :, 3])
            # cb2 / cn2
            V.tensor_scalar_mul(out=Q8[:, :, 5], in0=Q8[:, :, 2],
                                scalar1=2.0)
            G.tensor_scalar_mul(out=Q8[:, :, 7], in0=Q8[:, :, 4],
                                scalar1=2.0)

            # mask-select my core -> Ysel [1, 8] -> Y1 row 0
            Qm = sm.tile([1, 8, 8], F32, tag="Qm")
            V.tensor_mul(out=Qm[:, :, :], in0=Q8[:, :, :],
                         in1=msk[0:1, :].unsqueeze(2)
                         .broadcast_to([1, 8, 8]))
            Qf1 = sm.tile([1, 4, 8], F32, tag="Qf1")
            V.tensor_add(out=Qf1[:, :, :], in0=Qm[:, 0:4, :],
                         in1=Qm[:, 4:8, :])
            Qf2 = sm.tile([1, 2, 8], F32, tag="Qf2")
            V.tensor_add(out=Qf2[:, :, :], in0=Qf1[:, 0:2, :],
                         in1=Qf1[:, 2:4, :])
            Y1 = wk.tile([128, 8], F32, tag="Y1")
            G.memset(Y1[:, :], 0.0)
            V.tensor_add(out=Y1[0:1, :], in0=Qf2[:, 0, :],
                         in1=Qf2[:, 1, :])
            # broadcast partition-0 row to all partitions via PE
            ubp = ps1.tile([128, 8], F32, tag="pub")
            nc.tensor.matmul(out=ubp[:, :], lhsT=e0bc, rhs=Y1[:, :],
                             start=True, stop=True)
            ub = sm.tile([128, 8], F32, tag="ub")
            V.tensor_copy(out=ub[:, :], in_=ubp[:, :])

            # ---- phase 6: own-core slice extract + exclusive shift ----
            mR = wk.tile([128, 8, 4], F32, tag="mR")
            V.tensor_mul(
                out=mR[:, :, :], in0=Rcur[:, :, :],
                in1=msk[:, :].unsqueeze(2).broadcast_to([128, 8, 4]))
            s1 = wk.tile([128, 4, 4], F32, tag="s1")
            V.tensor_add(out=s1[:, :, :], in0=mR[:, 0:4, :],
                         in1=mR[:, 4:8, :])
            s2 = wk.tile([128, 2, 4], F32, tag="s2")
            V.tensor_add(out=s2[:, :, :], in0=s1[:, 0:2, :],
                         in1=s1[:, 2:4, :])
            SV = wk.tile([128, 2, 4], F32, tag="SV")  # [:,1,:] = inclusive
            V.tensor_add(out=SV[:, 1, :], in0=s2[:, 0, :], in1=s2[:, 1, :])
            prqt = ps4.tile([128, 4], F32, tag="sps")
            prq = prqt[:, :]
            nc.tensor.matmul(out=prq[:, :], lhsT=shsb[:, 0:128],
                             rhs=SV[:, 1, :], start=True, stop=True)
            V.tensor_add(out=SV[:, 0, :], in0=prq[:, :], in1=idsb[:, 0:4])

            # ---- phase 7: boundary outputs [128, 2] per quantity ----
            # SV cols: 0=p, 1=q, 2=u, 3=v  (side A=exclusive, B=inclusive)
            SV4 = SV.rearrange("p s (r c) -> p s r c", r=2)
            ubm = ub[:, 0:2]
            ubW = ub[:, 2:4]     # (cb, ck2)
            ubcn = ub[:, 4:5]
            ubcb2 = ub[:, 5:6]
            ubck2 = ub[:, 6:7]
            ubcn2 = ub[:, 7:8]

            # mu: alpha = p*m0 + q*m1 ; lam = u*m0 + v*m1
            tml = wk.tile([128, 2, 2, 2], F32, tag="tml")
            G.tensor_mul(out=tml[:, :, :, :], in0=SV4,
                         in1=ubm.unsqueeze(1).unsqueeze(2)
                         .broadcast_to([128, 2, 2, 2]))
            allam = wk.tile([128, 2, 2], F32, tag="allam")
            G.tensor_add(out=allam[:, :, :], in0=tml[:, :, :, 0],
                         in1=tml[:, :, :, 1])
            alpha = allam[:, :, 0]
            lam = allam[:, :, 1]

            q_ = SV[:, :, 1]
            v_ = SV[:, :, 3]
            PPQ = wk.tile([128, 2, 2], F32, tag="PPQ")   # (pp, pq)
            QQ2 = wk.tile([128, 2], F32, tag="QQ2")      # qq
            UUV = wk.tile([128, 2, 2], F32, tag="UUV")   # (uu, uv)
            VV2 = wk.tile([128, 2], F32, tag="VV2")      # vv
            PUV = wk.tile([128, 2, 2], F32, tag="PUV")   # (pu, pv)
            QUV = wk.tile([128, 2, 2], F32, tag="QUV")   # (qu, qv)
            V.tensor_mul(out=PPQ[:, :, :],
                         in0=SV[:, :, 0:1].broadcast_to([128, 2, 2]),
                         in1=SV[:, :, 0:2])
            V.tensor_mul(out=QQ2[:, :], in0=q_, in1=q_)
            G.tensor_mul(out=UUV[:, :, :],
                         in0=SV[:, :, 2:3].broadcast_to([128, 2, 2]),
                         in1=SV[:, :, 2:4])
            G.tensor_mul(out=VV2[:, :], in0=v_, in1=v_)
            V.tensor_mul(out=PUV[:, :, :],
                         in0=SV[:, :, 0:1].broadcast_to([128, 2, 2]),
                         in1=SV[:, :, 2:4])
            V.tensor_mul(out=QUV[:, :, :],
                         in0=SV[:, :, 1:2].broadcast_to([128, 2, 2]),
                         in1=SV[:, :, 2:4])

            bknt = wk.tile([128, 2, 3], F32, tag="bknt")  # beta, kappa, nu
            beta = bknt[:, :, 0]
            kap = bknt[:, :, 1]
            nu = bknt[:, :, 2]
            tb = wk.tile([128, 2, 2], F32, tag="tb")
            # beta = pp*cb + pq*ck2 + qq*cn
            V.tensor_mul(out=tb[:, :, :], in0=PPQ[:, :, :],
                         in1=ubW.unsqueeze(1).broadcast_to([128, 2, 2]))
            V.tensor_add(out=tb[:, :, 0], in0=tb[:, :, 0], in1=tb[:, :, 1])
            V.tensor_scalar_mul(out=tb[:, :, 1], in0=QQ2[:, :],
                                scalar1=ubcn)
            V.tensor_add(out=beta, in0=tb[:, :, 0], in1=tb[:, :, 1])
            # nu = uu*cb + uv*ck2 + vv*cn
            tn = wk.tile([128, 2, 2], F32, tag="tn")
            G.tensor_mul(out=tn[:, :, :], in0=UUV[:, :, :],
                         in1=ubW.unsqueeze(1).broadcast_to([128, 2, 2]))
            G.tensor_add(out=tn[:, :, 0], in0=tn[:, :, 0], in1=tn[:, :, 1])
            G.tensor_scalar_mul(out=tn[:, :, 1], in0=VV2[:, :],
                                scalar1=ubcn)
            G.tensor_add(out=nu, in0=tn[:, :, 0], in1=tn[:, :, 1])
            # kappa = pu*cb2 + (pv+qu)*ck2 + qv*cn2
            tk = wk.tile([128, 2, 2], F32, tag="tk")
            V.tensor_add(out=tk[:, :, 0], in0=PUV[:, :, 1],
                         in1=QUV[:, :, 0])
            V.tensor_scalar_mul(out=tk[:, :, 0], in0=tk[:, :, 0],
                                scalar1=ubck2)
            V.tensor_scalar_mul(out=tk[:, :, 1], in0=PUV[:, :, 0],
                                scalar1=ubcb2)
            V.tensor_add(out=tk[:, :, 0], in0=tk[:, :, 0], in1=tk[:, :, 1])
            V.tensor_scalar_mul(out=tk[:, :, 1], in0=QUV[:, :, 1],
                                scalar1=ubcn2)
            V.tensor_add(out=kap, in0=tk[:, :, 0], in1=tk[:, :, 1])
            # num = beta*lam^2 + nu*alpha^2 - 2*kappa*alpha*lam  (on G)
            nd = wk.tile([128, 2, 4], F32, tag="nd")
            G.tensor_mul(out=nd[:, :, 0], in0=lam, in1=lam)
            G.tensor_mul(out=nd[:, :, 0], in0=beta, in1=nd[:, :, 0])
            G.tensor_mul(out=nd[:, :, 1], in0=alpha, in1=alpha)
            G.tensor_mul(out=nd[:, :, 1], in0=nu, in1=nd[:, :, 1])
            G.tensor_add(out=nd[:, :, 0], in0=nd[:, :, 0], in1=nd[:, :, 1])
            G.tensor_mul(out=nd[:, :, 2], in0=alpha, in1=lam)
            G.tensor_mul(out=nd[:, :, 2], in0=kap, in1=nd[:, :, 2])
            G.tensor_scalar(out=nd[:, :, 2], in0=nd[:, :, 2], scalar1=-2.0,
                            scalar2=0.0, op0=ALU.mult, op1=ALU.add)
            G.tensor_add(out=nd[:, :, 0], in0=nd[:, :, 0], in1=nd[:, :, 2])
            # den = beta*nu - kappa^2  (on V)
            V.tensor_mul(out=nd[:, :, 1], in0=beta, in1=nu)
            V.tensor_mul(out=nd[:, :, 3], in0=kap, in1=kap)
            V.tensor_sub(out=nd[:, :, 1], in0=nd[:, :, 1], in1=nd[:, :, 3])
            nc.scalar.activation(out=nd[:, :, 0], in_=nd[:, :, 0],
                                 func=AF.Ln, bias=0.0, scale=1.0)
            nc.scalar.activation(out=nd[:, :, 1], in_=nd[:, :, 1],
                                 func=AF.Ln, bias=0.0, scale=1.0)
            lsnr = wk.tile([128, 2], F32, tag="lsnr")
            V.tensor_sub(out=lsnr[:, :], in0=nd[:, :, 0], in1=nd[:, :, 1])

            # ---- phase 8: lerp to fine grid ----
            # output ch order: alpha, lam, beta, kappa, kappa, nu, lsnr
            chans = [alpha, lam, beta, kap, kap, nu, lsnr[:, :]]
            out7 = wk.tile([CH, L, 7], F32, tag="out7")
            Dt = wk.tile([128, 7], F32, tag="Dt")
            for ci, chv in enumerate(chans):
                if ci == 4:
                    continue
                eng = G if ci in (1, 5) else V
                eng.tensor_sub(out=Dt[:, ci:ci + 1], in0=chv[:, 1:2],
                               in1=chv[:, 0:1])
            G.tensor_copy(out=Dt[:, 4:5], in_=Dt[:, 3:4])
            for ci, chv in enumerate(chans):
                eng = G if ci in (1, 4, 5) else V
                eng.tensor_scalar(out=out7[:, :, ci], in0=wp[:, :],
                                  scalar1=Dt[:, ci:ci + 1],
                                  scalar2=chv[:, 0:1],
                                  op0=ALU.mult, op1=ALU.add)

            nc.sync.dma_start(out=out_d[:, :],
                              in_=out7[:, :, :].rearrange("p l c -> p (l c)"))
    _hoist_matmul_waits(nc)
    return nc


_NC_CACHE = None
TRACE = False
LAST_EXEC_NS = None


def kernel(**inputs):
    global _NC_CACHE, LAST_EXEC_NS
    t = np.asarray(inputs["t_range"], np.float32)

    def f32(x):
        return np.ascontiguousarray(np.asarray(x, np.float32))

    w1cat = f32(inputs["fr_W1"])[:, 0]
    b1cat = f32(inputs["fr_b1"])
    w2t = np.ascontiguousarray(f32(inputs["fr_W2"]).T)   # [256 in, 256 out]
    b2cat = f32(inputs["fr_b2"])
    # swap output rows: fr2 row0 = r, row1 = f
    w3t = np.ascontiguousarray(f32(inputs["fr_W3"])[::-1, :].T)  # [256, 2]
    b3row = f32(inputs["fr_b3"])[::-1].copy()

    lbn = f32(inputs["log_beta_nu_zero"])
    beta0 = np.float32(np.exp(lbn[0]))
    nu0 = np.float32(np.exp(lbn[1]))
    rho0 = np.float32(1.0 / (1.0 + np.exp(-f32(inputs["log_rho_zero"])[0])))
    kappa0 = np.float32(rho0 * np.sqrt(beta0) * np.sqrt(nu0))

    # chain endpoints / midpoints / dt sums; partition p = chain 127-p,
    # chain-major flat layout: idx = p*8 + core
    ks = np.arange(NCORES)[None, :]
    cs = (CH - 1 - np.arange(CH))[:, None]     # reversed chain per partition
    a_idx = ks * PER + L * cs                  # [128 partitions, 8 cores]
    b_idx = np.minimum(a_idx + L, ks * PER + PER)
    t64 = np.asarray(t, np.float64)
    tmids = (0.5 * (t64[a_idx] + t64[b_idx])).astype(np.float32).reshape(-1)
    dtsum = (t64[b_idx] - t64[a_idx]).astype(np.float32).reshape(-1)

    w2p = np.zeros((128, 512), np.float32)
    for kt in range(2):
        w2p[:, kt * 256:(kt + 1) * 256] = w2t[kt * 128:(kt + 1) * 128, :]

    w3p = np.zeros((128, 4), np.float32)
    for kt in range(2):
        w3p[:, 2 * kt:2 * kt + 2] = w3t[kt * 128:(kt + 1) * 128, :]
    wsml = np.zeros((128, NWSML), np.float32)
    wsml[:, W_W1:W_W1 + 2] = w1cat.reshape(2, 128).T
    wsml[:, W_B1:W_B1 + 2] = b1cat.reshape(2, 128).T
    wsml[:, W_B2:W_B2 + 2] = b2cat.reshape(2, 128).T
    wsml[0:2, W_B3] = b3row
    wsml[0:2, W_AD] = [0.0, 1.0]

    mega = np.zeros((128, NMEGA), np.float32)
    mega[0, O_C0:O_C0 + 3] = [beta0, kappa0 / 2.0, nu0]
    for p in range(CH):
        c = CH - 1 - p
        n_real = min(L, PER - L * c)
        mega[p, O_WP:O_WP + L] = np.minimum(
            (np.arange(L) + 1.0) / n_real, 1.0)
    mega[0, O_E0:O_E0 + 128] = 1.0             # all-ones row 0
    for di in range(7):
        d = 1 << di
        mega[:, O_SH + di * 128:O_SH + (di + 1) * 128] = np.eye(
            128, k=-d, dtype=np.float32)
        for c in range(8):
            mega[128 - d:, O_ID + di * 32 + c * 4 + 0] = 1.0
            mega[128 - d:, O_ID + di * 32 + c * 4 + 3] = 1.0

    in_maps = []
    for c in range(NCORES):
        mgc = mega.copy()
        mgc[:, O_MSK + c] = 1.0
        in_maps.append({
            "tmids": tmids, "dtsum": dtsum,
            "w2p": w2p, "w3p": w3p, "wsml": wsml, "mega": mgc,
        })

    if _NC_CACHE is None:
        _NC_CACHE = build_program()
    nc = _NC_CACHE
    res = run_bass_kernel_spmd(nc, in_maps, core_ids=list(range(NCORES)),
                               trace=TRACE)
    LAST_EXEC_NS = res.exec_time_ns

    full = np.empty((T, 7), np.float32)
    lsnr0 = np.float32(np.log(nu0) - np.log(beta0 * nu0 - kappa0 ** 2))
    full[0] = [1.0, 0.0, beta0, kappa0, kappa0, nu0, lsnr0]
    for c in range(NCORES):
        o = np.asarray(res.results[c]["out"], np.float32).reshape(CH, L, 7)
        o = o[::-1].reshape(CH * L, 7)         # partition p = chain 127-p
        lo = c * PER
        full[lo + 1:lo + PER + 1] = o[:PER]
    return full


# revision 12
# speedup vs baseline: 1.0687x; 1.0687x over previous
"""Trainium2 Bass kernel for nn_ExpandedSchedule (ODE schedule solver).

Coarse-chain algorithm:
- The 6x6 per-step transform block-decomposes into a 2x2 block (alpha,lam)
  and a 3x3 block (beta,kappa,nu) that is exactly the symmetric square of
  the 2x2 (kappa = 2 * offdiag, C0_eff uses kappa0/2); component 5 and the
  whole g-MLP never reach the output and are dropped.
- f, r are smooth scalar functions of t and dt=5e-6, so one 2x2 transform
  per 196-step chain, T_c = I + (t_end - t_start) * M(t_mid), approximates
  the per-step Euler product to ~1e-9; the 7 outputs are computed at chain
  boundaries and linearly interpolated to the fine grid (curvature error
  ~1e-7).
- Every core evaluates the MLP at all 8*128 chain midpoints (1024 points)
  and computes every core's chain-product total locally: no collective.

Layout: chains are REVERSED onto partitions (partition p = chain 127-p,
shift matrices eye(k=-d)) so the core-total lands on partition 0, where
the whole 8-core carry (free-dim mini-scan over core totals, carry
quadratic forms, mask-select) runs as tiny single-partition DVE ops; one
PE matmul against an all-ones-row-0 matrix broadcasts the carry row to
all 128 partitions. No DRAM bounces for the carry. Small constants are
host-packed into one [128, NMEGA] parameter; tmids/dtsum are chain-major
so the transform-entry readback is strided-contiguous; activation tables
(Gelu, Ln) are prefetched so their loads overlap DMA / MLP phases.
"""

import sys
for _p in ("/opt/trn_rl_repo", "/root/.axon_site/_ro/trn_rl_repo"):
    if _p not in sys.path:
        sys.path.insert(0, _p)

import numpy as np

import concourse.bass as bass
import concourse.mybir as mybir
import concourse.tile as tile
from concourse.bass_utils import run_bass_kernel_spmd

F32 = mybir.dt.float32
F32R = mybir.dt.float32r
AF = mybir.ActivationFunctionType
ALU = mybir.AluOpType

T = 200001
N = T - 1
NCORES = 8
PER = N // NCORES            # 25000
CH = 128                     # chains per core (one per partition)
L = 196                      # fine steps per chain
NPTS = NCORES * CH           # 1024 MLP points (all cores' midpoints)

# wsml (early small weights param) column offsets
W_W3 = 0          # [128, 4]
W_W1 = 4          # [128, 2]
W_B1 = 6          # [128, 2]
W_B2 = 8          # [128, 2]
W_B3 = 10         # rows 0-1
W_AD = 11         # rows 0-1: (0, 1) adder for entry build
NWSML = 12

# mega-const column offsets
O_C0 = 0          # row 0: (beta0, kappa0/2, nu0)
O_MSK = 3         # [128, 8] one-hot my core (all rows)
O_WP = 11         # [128, 196] lerp weights (row p = chain 127-p)
O_E0 = 207        # [128, 128] all-ones row 0 (partition-0 broadcast)
O_SH = 335        # [128, 7*128] shift matrices eye(k=-2^di)
O_ID = 1231       # [128, 7*32] identity pads (rows >= 128-d)
NMEGA = 1455


def _combine22(nc, pool, A, B, out, eng0=None, eng1=None):
    """out = A @ B on flattened 2x2 entry views [P, nb, 4] (row-major)."""
    P, nb = A.shape[0], A.shape[1]
    eng0 = eng0 or nc.vector
    eng1 = eng1 or eng0
    A4 = A.rearrange("p b (i k) -> p b i k", i=2)
    B4 = B.rearrange("p b (k j) -> p b k j", k=2)
    O4 = out.rearrange("p b (i j) -> p b i j", i=2)
    ts = [pool.tile([128, nb, 2, 2], F32, tag=f"c22_{i}", name=f"c22_{i}")
          for i in range(2)]
    for k in range(2):
        ak = A4[:, :, :, k].unsqueeze(3).broadcast_to([P, nb, 2, 2])
        bk = B4[:, :, k, :].unsqueeze(2).broadcast_to([P, nb, 2, 2])
        (eng0 if k == 0 else eng1).tensor_mul(out=ts[k][:P, :, :, :],
                                              in0=ak, in1=bk)
    eng0.tensor_add(out=O4, in0=ts[0][:P, :, :, :], in1=ts[1][:P, :, :, :])


def _hoist_matmul_waits(nc):
    """This walrus codegen allows only one sync wait per engine instruction;
    move extra waits onto inserted same-engine NoOps just before it."""
    for fn in nc.m.functions:
        for bb in fn.blocks:
            new = []
            for ins in bb.instructions:
                si = getattr(ins, "sync_info", None)
                if (si is not None and si.on_wait and len(si.on_wait) > 1
                        and getattr(ins, "engine", None) is not None):
                    waits = list(si.on_wait)
                    si.on_wait = [waits.pop()]
                    for wi, w in enumerate(waits):
                        new.append(mybir.InstNoOp(
                            name=f"{ins.name}-wgate{wi}", engine=ins.engine,
                            ins=[], outs=[],
                            sync_info=mybir.SyncInfo(on_wait=[w],
                                                     on_update=[])))
                new.append(ins)
            bb.instructions = new


def build_program():
    nc = bass.Bass()

    tmid_d = nc.declare_dram_parameter("tmids", [NPTS], F32, isOutput=False)
    dts_d = nc.declare_dram_parameter("dtsum", [NPTS], F32, isOutput=False)
    w2_d = nc.declare_dram_parameter("w2p", [128, 512], F32, isOutput=False)
    w3_d = nc.declare_dram_parameter("w3p", [128, 4], F32, isOutput=False)
    ws_d = nc.declare_dram_parameter("wsml", [128, NWSML], F32,
                                     isOutput=False)
    mg_d = nc.declare_dram_parameter("mega", [128, NMEGA], F32,
                                     isOutput=False)
    out_d = nc.declare_dram_parameter("out", [CH, L * 7], F32, isOutput=True)

    with tile.TileContext(nc) as tc:
        with (
            tc.tile_pool(name="const", bufs=1) as cp,
            tc.tile_pool(name="dram", bufs=1, space="DRAM") as dp,
            tc.tile_pool(name="work", bufs=1) as wk,
            tc.tile_pool(name="sc2", bufs=1) as sc2,
            tc.tile_pool(name="lvl", bufs=2) as lvl,
            tc.tile_pool(name="ps", bufs=1, space="PSUM") as psp,
            tc.tile_pool(name="ps2", bufs=2, space="PSUM") as ps2,
            tc.tile_pool(name="ps4", bufs=2, space="PSUM") as ps4,
            tc.tile_pool(name="ps1", bufs=1, space="PSUM") as ps1,
            tc.tile_pool(name="sm", bufs=2) as sm,
        ):
            V = nc.vector
            G = nc.gpsimd

            # gelu-table prefetch before any DMA lands
            tiny = cp.tile([1, 1], F32)
            V.memset(tiny[:, :], 0.5)
            tinyo = cp.tile([1, 1], F32)
            nc.scalar.activation(out=tinyo[:, :], in_=tiny[:, :],
                                 func=AF.Gelu, bias=0.0, scale=1.0)

            # ---- input DMAs (MLP-critical first) ----
            tmb = cp.tile([128, NPTS], F32)
            nc.sync.dma_start(
                out=tmb[:, :],
                in_=tmid_d[:].unsqueeze(0).broadcast_to([128, NPTS]))
            wssb = cp.tile([128, NWSML], F32)
            nc.sync.dma_start(out=wssb[:, :], in_=ws_d[:, :])
            w2sb = cp.tile([128, 512], F32R)
            nc.sync.dma_start(out=w2sb[:, :],
                              in_=w2_d[:, :].bitcast(F32R))
            w3sb = cp.tile([128, 4], F32R)
            nc.sync.dma_start(out=w3sb[:, :],
                              in_=w3_d[:, :].bitcast(F32R))
            dt2 = cp.tile([2, NPTS], F32)
            nc.sync.dma_start(
                out=dt2[:, :],
                in_=dts_d[:].unsqueeze(0).broadcast_to([2, NPTS]))
            mg = cp.tile([128, NMEGA], F32)
            nc.sync.dma_start(out=mg[:, :], in_=mg_d[:, :])

            w1col = wssb[:, W_W1:W_W1 + 2]
            b1sb = wssb[:, W_B1:W_B1 + 2]
            b2sb = wssb[:, W_B2:W_B2 + 2]
            b3col = wssb[0:2, W_B3:W_B3 + 1]
            adcol = wssb[0:2, W_AD:W_AD + 1]
            cb0 = mg[0:1, O_C0 + 0:O_C0 + 1]
            ck0 = mg[0:1, O_C0 + 1:O_C0 + 2]
            cn0 = mg[0:1, O_C0 + 2:O_C0 + 3]
            msk = mg[:, O_MSK:O_MSK + 8]
            wp = mg[:, O_WP:O_WP + L]
            e0bc = mg[:, O_E0:O_E0 + 128]
            shsb = mg[:, O_SH:O_SH + 7 * 128]
            idsb = mg[:, O_ID:O_ID + 7 * 32]

            # ---- phase 1: fr-MLP at all 1024 chain midpoints ----
            # (w3 rows swapped host-side: fr2 row0 = r, row1 = f)
            h1 = [wk.tile([128, NPTS], F32R, tag=f"h1_{mi}",
                          name=f"h1_{mi}") for mi in range(2)]
            h2 = [wk.tile([128, NPTS], F32R, tag=f"h2_{mi}",
                          name=f"h2_{mi}") for mi in range(2)]
            fr2 = wk.tile([2, NPTS], F32, tag="fr2")
            HP = NPTS // 2
            for ti in range(2):
                sl = slice(ti * HP, (ti + 1) * HP)
                for mi in range(2):
                    nc.scalar.activation(out=h1[mi][:, sl], in_=tmb[:, sl],
                                         func=AF.Gelu,
                                         bias=b1sb[:, mi:mi + 1],
                                         scale=w1col[:, mi:mi + 1])
                for mi in range(2):
                    p2 = ps2.tile([128, HP], F32, tag=f"p2_{mi}",
                                  name=f"p2_{mi}_{ti}")
                    for kt in range(2):
                        lhs = w2sb[:, kt * 256 + mi * 128:
                                   kt * 256 + (mi + 1) * 128]
                        nc.tensor.matmul(out=p2[:, :], lhsT=lhs,
                                         rhs=h1[kt][:, sl],
                                         start=(kt == 0), stop=(kt == 1))
                    nc.scalar.activation(out=h2[mi][:, sl], in_=p2[:, :],
                                         func=AF.Gelu,
                                         bias=b2sb[:, mi:mi + 1], scale=1.0)
                p3 = ps2.tile([2, HP], F32, tag="p2_0", name=f"p3_{ti}")
                for kt in range(2):
                    nc.tensor.matmul(out=p3[:, :],
                                     lhsT=w3sb[:, 2 * kt:2 * kt + 2],
                                     rhs=h2[kt][:, sl],
                                     start=(kt == 0), stop=(kt == 1))
                nc.vector.tensor_scalar_add(out=fr2[:, sl], in0=p3[:, :],
                                            scalar1=b3col[:, :])

            # (Ln-table prefetch removed: bisecting precision regression)

            # ---- phase 2: entry rows: row0 = -dts*r = T01,
            #      row1 = 1 - dts*f = T11 ----
            dtfr = wk.tile([2, NPTS], F32, tag="dtfr")
            V.tensor_mul(out=dtfr[:, :], in0=dt2[:, :], in1=fr2[:, :])
            ent = wk.tile([2, NPTS], F32, tag="ent")
            V.tensor_scalar(out=ent[:, :], in0=dtfr[:, :],
                            scalar1=-1.0, scalar2=adcol[:, :],
                            op0=ALU.mult, op1=ALU.add)
            entd = dp.tile([2, NPTS], F32)
            nc.sync.dma_start(out=entd[:, :], in_=ent[:, :])

            # ---- phase 3: Tc [128, 8 cores, 4]; partition p = chain 127-p
            #      (host already stores tmids/dtsum with reversed chains) ----
            Tc = wk.tile([128, 8, 4], F32, tag="Tc")
            Tcv = Tc[:, :, :]
            Tc4 = Tc.rearrange("p k (a b) -> p k a b", a=2)
            V.memset(Tcv[:, :, 0], 1.0)
            nc.sync.dma_start(
                out=Tc4[:, :, :, 1],
                in_=entd[:, :].rearrange("e (c k) -> c k e", k=8))
            nc.sync.dma_start(
                out=Tcv[:, :, 2],
                in_=dts_d[:].rearrange("(c k) -> c k", k=8))

            # ---- phase 4: joint 7-level Hillis-Steele over partitions ----
            # R_p <- R_p @ R_{p+d} (shift matrices eye(k=-d))
            Rcur = Tc
            for di in range(7):
                pr = psp.tile([128, 32], F32, tag="spr")
                nc.tensor.matmul(out=pr[:, :],
                                 lhsT=shsb[:, di * 128:(di + 1) * 128],
                                 rhs=Rcur[:, :, :].rearrange(
                                     "p k e -> p (k e)"),
                                 start=True, stop=True)
                Bv = lvl.tile([128, 8, 4], F32, tag="Bv", name=f"Bv{di}")
                V.tensor_add(
                    out=Bv[:, :, :],
                    in0=pr[:, :].rearrange("p (k e) -> p k e", k=8),
                    in1=idsb[:, di * 32:(di + 1) * 32].rearrange(
                        "p (k e) -> p k e", k=8))
                Rn = lvl.tile([128, 8, 4], F32, tag="R", name=f"R{di}")
                _combine22(nc, sc2, Rcur[:, :, :], Bv[:, :, :], Rn[:, :, :],
                           eng0=V, eng1=V)
                Rcur = Rn

            # ---- phase 5: carry, entirely on partition 0 ----
            # core totals: Rcur[0, k, :] = full product of core k's chains
            Z = Rcur[0:1, :, :]                     # [1, 8, 4]
            for di in range(3):
                d = 1 << di
                Zn = sm.tile([1, 8, 4], F32, tag="Zn", name=f"Zn{di}")
                G.tensor_copy(out=Zn[:, 0:d, :], in_=Z[:, 0:d, :])
                _combine22(nc, sc2, Z[:, d:8, :], Z[:, 0:8 - d, :],
                           Zn[:, d:8, :], eng0=V)
                Z = Zn[:, :, :]
            # exclusive over cores: Kx[c] = product of cores < c; Kx[0] = I
            Kx = sm.tile([1, 8, 4], F32, tag="Kx")
            V.tensor_copy(out=Kx[:, 1:8, :], in_=Z[:, 0:7, :])
            V.memset(Kx[:, 0, :], 0.0)
            V.memset(Kx[:, 0, 0::3], 1.0)

            # carry quantities Q8 [1, 8 cores, 8]:
            # 0=m0 1=m1 2=cb 3=ck2 4=cn 5=cb2 6=ck2 7=cn2
            kxp = Kx[:, :, 0:1]
            KP = sm.tile([1, 8, 4], F32, tag="KP")   # (pp, pq, pu, pv)
            KQ = sm.tile([1, 8, 4], F32, tag="KQ")   # (qp, qq, qu, qv)
            KU = sm.tile([1, 8, 4], F32, tag="KU")   # (up, uq, uu, uv)
            KV = sm.tile([1, 8, 4], F32, tag="KV")   # (vp, vq, vu, vv)
            V.tensor_mul(out=KP[:, :, :], in0=kxp.broadcast_to([1, 8, 4]),
                         in1=Kx[:, :, :])
            G.tensor_mul(out=KQ[:, :, :],
                         in0=Kx[:, :, 1:2].broadcast_to([1, 8, 4]),
                         in1=Kx[:, :, :])
            V.tensor_mul(out=KU[:, :, :],
                         in0=Kx[:, :, 2:3].broadcast_to([1, 8, 4]),
                         in1=Kx[:, :, :])
            G.tensor_mul(out=KV[:, :, :],
                         in0=Kx[:, :, 3:4].broadcast_to([1, 8, 4]),
                         in1=Kx[:, :, :])
            Q8 = sm.tile([1, 8, 8], F32, tag="Q8")
            V.tensor_copy(out=Q8[:, :, 0], in_=Kx[:, :, 0])
            V.tensor_copy(out=Q8[:, :, 1], in_=Kx[:, :, 2])
            tq = sm.tile([1, 8, 4], F32, tag="tq")
            # cb = pp*cb0 + 2*pq*ck0 + qq*cn0
            V.tensor_scalar_mul(out=tq[:, :, 0], in0=KP[:, :, 0],
                                scalar1=cb0)
            V.tensor_scalar(out=tq[:, :, 1], in0=KP[:, :, 1], scalar1=ck0,
                            scalar2=2.0, op0=ALU.mult, op1=ALU.mult)
            V.tensor_add(out=tq[:, :, 0], in0=tq[:, :, 0], in1=tq[:, :, 1])
            V.tensor_scalar_mul(out=tq[:, :, 1], in0=KQ[:, :, 1],
                                scalar1=cn0)
            V.tensor_add(out=Q8[:, :, 2], in0=tq[:, :, 0], in1=tq[:, :, 1])
            # cn = uu*cb0 + 2*uv*ck0 + vv*cn0
            G.tensor_scalar_mul(out=tq[:, :, 2], in0=KU[:, :, 2],
                                scalar1=cb0)
            G.tensor_scalar(out=tq[:, :, 3], in0=KU[:, :, 3], scalar1=ck0,
                            scalar2=2.0, op0=ALU.mult, op1=ALU.mult)
            G.tensor_add(out=tq[:, :, 2], in0=tq[:, :, 2], in1=tq[:, :, 3])
            G.tensor_scalar_mul(out=tq[:, :, 3], in0=KV[:, :, 3],
                                scalar1=cn0)
            G.tensor_add(out=Q8[:, :, 4], in0=tq[:, :, 2], in1=tq[:, :, 3])
            # ck2 = 2*(pu*cb0 + (pv+qu)*ck0 + qv*cn0)
            tk8 = sm.tile([1, 8, 2], F32, tag="tk8")
            V.tensor_scalar(out=tk8[:, :, 0], in0=KP[:, :, 2], scalar1=cb0,
                            scalar2=2.0, op0=ALU.mult, op1=ALU.mult)
            V.tensor_add(out=tk8[:, :, 1], in0=KP[:, :, 3], in1=KQ[:, :, 2])
            V.tensor_scalar(out=tk8[:, :, 1], in0=tk8[:, :, 1], scalar1=ck0,
                            scalar2=2.0, op0=ALU.mult, op1=ALU.mult)
            V.tensor_add(out=tk8[:, :, 0], in0=tk8[:, :, 0],
                         in1=tk8[:, :, 1])
            V.tensor_scalar(out=tk8[:, :, 1], in0=KQ[:, :, 3], scalar1=cn0,
                            scalar2=2.0, op0=ALU.mult, op1=ALU.mult)
            V.tensor_add(out=Q8[:, :, 3], in0=tk8[:, :, 0],
                         in1=tk8[:, :, 1])
            V.tensor_copy(out=Q8[:, :, 6], in_=Q8[:, :, 3])
            # cb2 / cn2
            V.tensor_scalar_mul(out=Q8[:, :, 5], in0=Q8[:, :, 2],
                                scalar1=2.0)
            G.tensor_scalar_mul(out=Q8[:, :, 7], in0=Q8[:, :, 4],
                                scalar1=2.0)

            # mask-select my core -> Ysel [1, 8] -> Y1 row 0
            Qm = sm.tile([1, 8, 8], F32, tag="Qm")
            V.tensor_mul(out=Qm[:, :, :], in0=Q8[:, :, :],
                         in1=msk[0:1, :].unsqueeze(2)
                         .broadcast_to([1, 8, 8]))
            Qf1 = sm.tile([1, 4, 8], F32, tag="Qf1")
            V.tensor_add(out=Qf1[:, :, :], in0=Qm[:, 0:4, :],
                         in1=Qm[:, 4:8, :])
            Qf2 = sm.tile([1, 2, 8], F32, tag="Qf2")
            V.tensor_add(out=Qf2[:, :, :], in0=Qf1[:, 0:2, :],
                         in1=Qf1[:, 2:4, :])
            Y1 = wk.tile([128, 8], F32, tag="Y1")
            G.memset(Y1[:, :], 0.0)
            V.tensor_add(out=Y1[0:1, :], in0=Qf2[:, 0, :],
                         in1=Qf2[:, 1, :])
            # broadcast partition-0 row to all partitions via PE
            ubp = ps1.tile([128, 8], F32, tag="pub")
            nc.tensor.matmul(out=ubp[:, :], lhsT=e0bc, rhs=Y1[:, :],
                             start=True, stop=True)
            ub = sm.tile([128, 8], F32, tag="ub")
            V.tensor_copy(out=ub[:, :], in_=ubp[:, :])

            # ---- phase 6: own-core slice extract + exclusive shift ----
            mR = wk.tile([128, 8, 4], F32, tag="mR")
            V.tensor_mul(
                out=mR[:, :, :], in0=Rcur[:, :, :],
                in1=msk[:, :].unsqueeze(2).broadcast_to([128, 8, 4]))
            s1 = wk.tile([128, 4, 4], F32, tag="s1")
            V.tensor_add(out=s1[:, :, :], in0=mR[:, 0:4, :],
                         in1=mR[:, 4:8, :])
            s2 = wk.tile([128, 2, 4], F32, tag="s2")
            V.tensor_add(out=s2[:, :, :], in0=s1[:, 0:2, :],
                         in1=s1[:, 2:4, :])
            SV = wk.tile([128, 2, 4], F32, tag="SV")  # [:,1,:] = inclusive
            V.tensor_add(out=SV[:, 1, :], in0=s2[:, 0, :], in1=s2[:, 1, :])
            prqt = ps4.tile([128, 4], F32, tag="sps")
            prq = prqt[:, :]
            nc.tensor.matmul(out=prq[:, :], lhsT=shsb[:, 0:128],
                             rhs=SV[:, 1, :], start=True, stop=True)
            V.tensor_add(out=SV[:, 0, :], in0=prq[:, :], in1=idsb[:, 0:4])

            # ---- phase 7: boundary outputs [128, 2] per quantity ----
            # SV cols: 0=p, 1=q, 2=u, 3=v  (side A=exclusive, B=inclusive)
            SV4 = SV.rearrange("p s (r c) -> p s r c", r=2)
            ubm = ub[:, 0:2]
            ubW = ub[:, 2:4]     # (cb, ck2)
            ubcn = ub[:, 4:5]
            ubcb2 = ub[:, 5:6]
            ubck2 = ub[:, 6:7]
            ubcn2 = ub[:, 7:8]

            # mu: alpha = p*m0 + q*m1 ; lam = u*m0 + v*m1
            tml = wk.tile([128, 2, 2, 2], F32, tag="tml")
            G.tensor_mul(out=tml[:, :, :, :], in0=SV4,
                         in1=ubm.unsqueeze(1).unsqueeze(2)
                         .broadcast_to([128, 2, 2, 2]))
            allam = wk.tile([128, 2, 2], F32, tag="allam")
            G.tensor_add(out=allam[:, :, :], in0=tml[:, :, :, 0],
                         in1=tml[:, :, :, 1])
            alpha = allam[:, :, 0]
            lam = allam[:, :, 1]

            q_ = SV[:, :, 1]
            v_ = SV[:, :, 3]
            PPQ = wk.tile([128, 2, 2], F32, tag="PPQ")   # (pp, pq)
            QQ2 = wk.tile([128, 2], F32, tag="QQ2")      # qq
            UUV = wk.tile([128, 2, 2], F32, tag="UUV")   # (uu, uv)
            VV2 = wk.tile([128, 2], F32, tag="VV2")      # vv
            PUV = wk.tile([128, 2, 2], F32, tag="PUV")   # (pu, pv)
            QUV = wk.tile([128, 2, 2], F32, tag="QUV")   # (qu, qv)
            V.tensor_mul(out=PPQ[:, :, :],
                         in0=SV[:, :, 0:1].broadcast_to([128, 2, 2]),
                         in1=SV[:, :, 0:2])
            V.tensor_mul(out=QQ2[:, :], in0=q_, in1=q_)
            G.tensor_mul(out=UUV[:, :, :],
                         in0=SV[:, :, 2:3].broadcast_to([128, 2, 2]),
                         in1=SV[:, :, 2:4])
            G.tensor_mul(out=VV2[:, :], in0=v_, in1=v_)
            V.tensor_mul(out=PUV[:, :, :],
                         in0=SV[:, :, 0:1].broadcast_to([128, 2, 2]),
                         in1=SV[:, :, 2:4])
            V.tensor_mul(out=QUV[:, :, :],
                         in0=SV[:, :, 1:2].broadcast_to([128, 2, 2]),
                         in1=SV[:, :, 2:4])

            bknt = wk.tile([128, 2, 3], F32, tag="bknt")  # beta, kappa, nu
            beta = bknt[:, :, 0]
            kap = bknt[:, :, 1]
            nu = bknt[:, :, 2]
            tb = wk.tile([128, 2, 2], F32, tag="tb")
            # beta = pp*cb + pq*ck2 + qq*cn
            V.tensor_mul(out=tb[:, :, :], in0=PPQ[:, :, :],
                         in1=ubW.unsqueeze(1).broadcast_to([128, 2, 2]))
            V.tensor_add(out=tb[:, :, 0], in0=tb[:, :, 0], in1=tb[:, :, 1])
            V.tensor_scalar_mul(out=tb[:, :, 1], in0=QQ2[:, :],
                                scalar1=ubcn)
            V.tensor_add(out=beta, in0=tb[:, :, 0], in1=tb[:, :, 1])
            # nu = uu*cb + uv*ck2 + vv*cn
            tn = wk.tile([128, 2, 2], F32, tag="tn")
            G.tensor_mul(out=tn[:, :, :], in0=UUV[:, :, :],
                         in1=ubW.unsqueeze(1).broadcast_to([128, 2, 2]))
            G.tensor_add(out=tn[:, :, 0], in0=tn[:, :, 0], in1=tn[:, :, 1])
            G.tensor_scalar_mul(out=tn[:, :, 1], in0=VV2[:, :],
                                scalar1=ubcn)
            G.tensor_add(out=nu, in0=tn[:, :, 0], in1=tn[:, :, 1])
            # kappa = pu*cb2 + (pv+qu)*ck2 + qv*cn2
            tk = wk.tile([128, 2, 2], F32, tag="tk")
            V.tensor_add(out=tk[:, :, 0], in0=PUV[:, :, 1],
                         in1=QUV[:, :, 0])
            V.tensor_scalar_mul(out=tk[:, :, 0], in0=tk[:, :, 0],
                                scalar1=ubck2)
            V.tensor_scalar_mul(out=tk[:, :, 1], in0=PUV[:, :, 0],
                                scalar1=ubcb2)
            V.tensor_add(out=tk[:, :, 0], in0=tk[:, :, 0], in1=tk[:, :, 1])
            V.tensor_scalar_mul(out=tk[:, :, 1], in0=QUV[:, :, 1],
                                scalar1=ubcn2)
            V.tensor_add(out=kap, in0=tk[:, :, 0], in1=tk[:, :, 1])
            # num = beta*lam^2 + nu*alpha^2 - 2*kappa*alpha*lam  (on G)
            nd = wk.tile([128, 2, 4], F32, tag="nd")
            G.tensor_mul(out=nd[:, :, 0], in0=lam, in1=lam)
            G.tensor_mul(out=nd[:, :, 0], in0=beta, in1=nd[:, :, 0])
            G.tensor_mul(out=nd[:, :, 1], in0=alpha, in1=alpha)
            G.tensor_mul(out=nd[:, :, 1], in0=nu, in1=nd[:, :, 1])
            G.tensor_add(out=nd[:, :, 0], in0=nd[:, :, 0], in1=nd[:, :, 1])
            G.tensor_mul(out=nd[:, :, 2], in0=alpha, in1=lam)
            G.tensor_mul(out=nd[:, :, 2], in0=kap, in1=nd[:, :, 2])
            G.tensor_scalar(out=nd[:, :, 2], in0=nd[:, :, 2], scalar1=-2.0,
                            scalar2=0.0, op0=ALU.mult, op1=ALU.add)
            G.tensor_add(out=nd[:, :, 0], in0=nd[:, :, 0], in1=nd[:, :, 2])
            # den = beta*nu - kappa^2  (on V)
            V.tensor_mul(out=nd[:, :, 1], in0=beta, in1=nu)
            V.tensor_mul(out=nd[:, :, 3], in0=kap, in1=kap)
            V.tensor_sub(out=nd[:, :, 1], in0=nd[:, :, 1], in1=nd[:, :, 3])
            nc.scalar.activation(out=nd[:, :, 0], in_=nd[:, :, 0],
                                 func=AF.Ln, bias=0.0, scale=1.0)
            nc.scalar.activation(out=nd[:, :, 1], in_=nd[:, :, 1],
                                 func=AF.Ln, bias=0.0, scale=1.0)
            lsnr = wk.tile([128, 2], F32, tag="lsnr")
            V.tensor_sub(out=lsnr[:, :], in0=nd[:, :, 0], in1=nd[:, :, 1])

            # ---- phase 8: lerp to fine grid ----
            # output ch order: alpha, lam, beta, kappa, kappa, nu, lsnr
            chans = [alpha, lam, beta, kap, kap, nu, lsnr[:, :]]
            out7 = wk.tile([CH, L, 7], F32, tag="out7")
            Dt = wk.tile([128, 7], F32, tag="Dt")
            for ci, chv in enumerate(chans):
                if ci == 4:
                    continue
                eng = G if ci in (1, 5) else V
                eng.tensor_sub(out=Dt[:, ci:ci + 1], in0=chv[:, 1:2],
                               in1=chv[:, 0:1])
            G.tensor_copy(out=Dt[:, 4:5], in_=Dt[:, 3:4])
            for ci, chv in enumerate(chans):
                eng = G if ci in (1, 4, 5) else V
                eng.tensor_scalar(out=out7[:, :, ci], in0=wp[:, :],
                                  scalar1=Dt[:, ci:ci + 1],
                                  scalar2=chv[:, 0:1],
                                  op0=ALU.mult, op1=ALU.add)

            nc.sync.dma_start(out=out_d[:, :],
                              in_=out7[:, :, :].rearrange("p l c -> p (l c)"))
    _hoist_matmul_waits(nc)
    return nc


_NC_CACHE = None
TRACE = False
LAST_EXEC_NS = None


def kernel(**inputs):
    global _NC_CACHE, LAST_EXEC_NS
    t = np.asarray(inputs["t_range"], np.float32)

    def f32(x):
        return np.ascontiguousarray(np.asarray(x, np.float32))

    w1cat = f32(inputs["fr_W1"])[:, 0]
    b1cat = f32(inputs["fr_b1"])
    w2t = np.ascontiguousarray(f32(inputs["fr_W2"]).T)   # [256 in, 256 out]
    b2cat = f32(inputs["fr_b2"])
    # swap output rows: fr2 row0 = r, row1 = f
    w3t = np.ascontiguousarray(f32(inputs["fr_W3"])[::-1, :].T)  # [256, 2]
    b3row = f32(inputs["fr_b3"])[::-1].copy()

    lbn = f32(inputs["log_beta_nu_zero"])
    beta0 = np.float32(np.exp(lbn[0]))
    nu0 = np.float32(np.exp(lbn[1]))
    rho0 = np.float32(1.0 / (1.0 + np.exp(-f32(inputs["log_rho_zero"])[0])))
    kappa0 = np.float32(rho0 * np.sqrt(beta0) * np.sqrt(nu0))

    # chain endpoints / midpoints / dt sums; partition p = chain 127-p,
    # chain-major flat layout: idx = p*8 + core
    ks = np.arange(NCORES)[None, :]
    cs = (CH - 1 - np.arange(CH))[:, None]     # reversed chain per partition
    a_idx = ks * PER + L * cs                  # [128 partitions, 8 cores]
    b_idx = np.minimum(a_idx + L, ks * PER + PER)
    t64 = np.asarray(t, np.float64)
    tmids = (0.5 * (t64[a_idx] + t64[b_idx])).astype(np.float32).reshape(-1)
    dtsum = (t64[b_idx] - t64[a_idx]).astype(np.float32).reshape(-1)

    w2p = np.zeros((128, 512), np.float32)
    for kt in range(2):
        w2p[:, kt * 256:(kt + 1) * 256] = w2t[kt * 128:(kt + 1) * 128, :]

    w3p = np.zeros((128, 4), np.float32)
    for kt in range(2):
        w3p[:, 2 * kt:2 * kt + 2] = w3t[kt * 128:(kt + 1) * 128, :]
    wsml = np.zeros((128, NWSML), np.float32)
    wsml[:, W_W1:W_W1 + 2] = w1cat.reshape(2, 128).T
    wsml[:, W_B1:W_B1 + 2] = b1cat.reshape(2, 128).T
    wsml[:, W_B2:W_B2 + 2] = b2cat.reshape(2, 128).T
    wsml[0:2, W_B3] = b3row
    wsml[0:2, W_AD] = [0.0, 1.0]

    mega = np.zeros((128, NMEGA), np.float32)
    mega[0, O_C0:O_C0 + 3] = [beta0, kappa0 / 2.0, nu0]
    for p in range(CH):
        c = CH - 1 - p
        n_real = min(L, PER - L * c)
        mega[p, O_WP:O_WP + L] = np.minimum(
            (np.arange(L) + 1.0) / n_real, 1.0)
    mega[0, O_E0:O_E0 + 128] = 1.0             # all-ones row 0
    for di in range(7):
        d = 1 << di
        mega[:, O_SH + di * 128:O_SH + (di + 1) * 128] = np.eye(
            128, k=-d, dtype=np.float32)
        for c in range(8):
            mega[128 - d:, O_ID + di * 32 + c * 4 + 0] = 1.0
            mega[128 - d:, O_ID + di * 32 + c * 4 + 3] = 1.0

    in_maps = []
    for c in range(NCORES):
        mgc = mega.copy()
        mgc[:, O_MSK + c] = 1.0
        in_maps.append({
            "tmids": tmids, "dtsum": dtsum,
            "w2p": w2p, "w3p": w3p, "wsml": wsml, "mega": mgc,
        })

    if _NC_CACHE is None:
        _NC_CACHE = build_program()
    nc = _NC_CACHE
    res = run_bass_kernel_spmd(nc, in_maps, core_ids=list(range(NCORES)),
                               trace=TRACE)
    LAST_EXEC_NS = res.exec_time_ns

    full = np.empty((T, 7), np.float32)
    lsnr0 = np.float32(np.log(nu0) - np.log(beta0 * nu0 - kappa0 ** 2))
    full[0] = [1.0, 0.0, beta0, kappa0, kappa0, nu0, lsnr0]
    for c in range(NCORES):
        o = np.asarray(res.results[c]["out"], np.float32).reshape(CH, L, 7)
        o = o[::-1].reshape(CH * L, 7)         # partition p = chain 127-p
        lo = c * PER
        full[lo + 1:lo + PER + 1] = o[:PER]
    return full
